# revision 1
# baseline (speedup 1.0000x reference)
"""Blockwise-dropout GEMM (DropoutMM) for 8x Trainium2 NeuronCores.

Computes: out = (x * expand(block_mask) / (1-p)) @ weight.T
  x: [8192, 4096] f32, weight: [4096, 4096] f32, block_mask: [64, 32] i32
  (128x128 blocks of x are kept/dropped per block_mask)

Strategy:
  - Shard N (weight out_features) across the 8 cores: each core computes the
    full 8192 rows x 512 columns of the output. The block mask pattern seen by
    every core is then identical, so a single SPMD program serves all cores.
  - The mask is known when kernel() is called, so dropped 128x128 blocks are
    skipped at trace time: the PE only runs matmuls for kept blocks (~50%).
  - The 1/(1-p) rescale is folded into the weight on the host.
  - Matmuls run in float32r (TF32-like): full PE rate with fp32 data.
  - Host prep: x blocks are gathered (kept only) and transposed to [K, M]
    block layout; weight is transposed to [K, N] layout and sliced per core.
"""

import os
import sys

import numpy as np

for _p in ("/opt/trn_rl_repo", "/root/.axon_site/_ro/trn_rl_repo"):
    if os.path.isdir(_p) and _p not in sys.path:
        sys.path.insert(0, _p)

BLOCK = 128
P_DROP = 0.1
N_CORES = 8

# Filled in by the last kernel() call when KERNEL_TRACE=1 (used by test.py).
LAST_RUN_INFO = {}


def _build_program(kept, counts, nsh, kb_blocks):
    """Trace + compile the per-core Bass program (identical on all cores).

    kept: list (len mb) of arrays of kept kb indices per block-row
    counts: kept-lengths; nsh: per-core N shard width; kb_blocks: K // BLOCK
    """
    from concourse import bacc
    import concourse.mybir as mybir
    import concourse.tile as tile

    P = BLOCK
    mb = len(kept)
    tot = int(sum(counts))
    cmax = max(1, int(max(counts)))

    nc = bacc.Bacc("TRN2", target_bir_lowering=False)
    XC = nc.dram_tensor("XC", [P, tot * P], mybir.dt.float32r, kind="ExternalInput")
    WS = nc.dram_tensor(
        "WS", [kb_blocks, P, nsh], mybir.dt.float32r, kind="ExternalInput"
    )
    OUT = nc.dram_tensor("OUT", [mb, P, nsh], mybir.dt.float32, kind="ExternalOutput")

    # Order the weight-block preload DMAs by first use so early slots start asap.
    worder = []
    seen = set()
    for s in range(mb):
        for b in kept[s]:
            if b not in seen:
                seen.add(b)
                worder.append(int(b))
    for b in range(kb_blocks):
        if b not in seen:
            worder.append(b)

    with tile.TileContext(nc) as tc:
        with (
            tc.tile_pool(name="wpool", bufs=1) as wpool,
            tc.tile_pool(name="xpool", bufs=4) as xpool,
            tc.tile_pool(name="opool", bufs=4) as opool,
            tc.tile_pool(name="psum", bufs=8, space="PSUM") as psum,
        ):
            w_res = wpool.tile([P, kb_blocks, nsh], mybir.dt.float32r, tag="w")
            for b in worder:
                nc.sync.dma_start(out=w_res[:, b], in_=WS[b])

            off = 0
            for s in range(mb):
                c = int(counts[s])
                ot = opool.tile([P, nsh], mybir.dt.float32, tag="o")
                if c == 0:
                    nc.any.memset(ot, 0.0)
                else:
                    xt = xpool.tile([P, cmax * P], mybir.dt.float32r, tag="x")
                    nc.sync.dma_start(
                        out=xt[:, : c * P], in_=XC[:, off * P : (off + c) * P]
                    )
                    pt = psum.tile([P, nsh], mybir.dt.float32, tag="ps")
                    for j, b in enumerate(kept[s]):
                        nc.tensor.matmul(
                            pt,
                            xt[:, j * P : (j + 1) * P],
                            w_res[:, int(b)],
                            start=(j == 0),
                            stop=(j == c - 1),
                        )
                    nc.vector.tensor_copy(out=ot, in_=pt)
                nc.sync.dma_start(out=OUT[s], in_=ot)
                off += c
    nc.compile()
    return nc


def kernel(x, weight, block_mask):
    from concourse.bass_utils import run_bass_kernel_spmd

    x = np.ascontiguousarray(x, dtype=np.float32)
    weight = np.ascontiguousarray(weight, dtype=np.float32)
    bm = np.asarray(block_mask)

    M, K = x.shape
    N = weight.shape[0]
    assert weight.shape == (N, K)
    mb, kb_blocks = bm.shape
    assert mb * BLOCK == M and kb_blocks * BLOCK == K
    assert N % N_CORES == 0
    nsh = N // N_CORES
    P = BLOCK

    kept = [np.flatnonzero(bm[s]) for s in range(mb)]
    counts = np.array([len(k) for k in kept], dtype=np.int64)
    tot = int(counts.sum())

    scale = np.float32(1.0 / (1.0 - P_DROP))

    if tot == 0:
        return np.zeros((M, N), dtype=np.float32)

    # ---- host data prep ----
    # XC: [128, tot*128]; for slot s, kept block j (mask col b):
    #   XC[k, (off_s+j)*128 + m] = x[s*128 + m, b*128 + k]
    x4 = x.reshape(mb, P, kb_blocks, P)  # [s, m, b, k]
    XC_np = np.empty((P, tot * P), dtype=np.float32)
    off = 0
    for s in range(mb):
        c = int(counts[s])
        if c == 0:
            continue
        blk = x4[s][:, kept[s], :]  # [m, c, k]
        XC_np[:, off * P : (off + c) * P] = np.ascontiguousarray(
            blk.transpose(2, 1, 0)
        ).reshape(P, c * P)
        off += c

    # WS per core: [kb, 128, nsh] with WS[b, k, n] = weight[c0+n, b*128+k] * scale
    wT = np.ascontiguousarray(weight.T) * scale  # [K, N]
    w4 = wT.reshape(kb_blocks, P, N)
    ws_list = [
        np.ascontiguousarray(w4[:, :, c * nsh : (c + 1) * nsh])
        for c in range(N_CORES)
    ]

    # ---- build + run ----
    nc = _build_program(kept, counts, nsh, kb_blocks)

    in_maps = [{"XC": XC_np, "WS": ws_list[c]} for c in range(N_CORES)]

    trace = os.environ.get("KERNEL_TRACE", "0") == "1"
    run_kwargs = {}
    if trace:
        try:
            sys.path.insert(0, os.path.dirname(os.path.abspath(__file__)))
            import ntff_shim  # noqa: F401

            run_kwargs = {
                "trace": True,
                "trace_cores": [int(os.environ.get("KERNEL_TRACE_CORE", "0"))],
            }
        except Exception as e:  # profiling is best-effort
            print(f"kernel: tracing unavailable ({e})", file=sys.stderr)

    res = run_bass_kernel_spmd(
        nc, in_maps, core_ids=list(range(N_CORES)), **run_kwargs
    )

    LAST_RUN_INFO.clear()
    LAST_RUN_INFO.update(
        exec_time_ns=res.exec_time_ns,
        mean_exec_time_ns=res.mean_exec_time_ns,
        trace=res.instructions_and_trace[1] if res.instructions_and_trace else None,
        profile_json=res.profile_json,
    )

    out = np.concatenate(
        [res.results[c]["OUT"].reshape(M, nsh) for c in range(N_CORES)], axis=1
    )
    return out
